# revision 14
# baseline (speedup 1.0000x reference)
"""Trainium2 Bass kernel for a 2-layer leaky-integrate-and-fire SNN.

Model (per timestep t, snnTorch Leaky with reset-by-subtraction):
    cur1 = x_t @ w1.T + b1
    mem1 = beta*mem1_prev + cur1 - (mem1_prev > 1)          # threshold 1.0
    spk1 = (mem1 > 1)
    cur2 = spk1 @ w2.T + b2
    mem2 = beta*mem2_prev + cur2 - (mem2_prev > 1)
    spk2 = (mem2 > 1)
Outputs: spk2 (B,T,O) and mem2 (B,T,O).

Strategy (data-parallel over batch, 16 rows per core):
  * cur1 for ALL timesteps is a feed-forward GEMM (the recurrence is only
    elementwise), computed in t-blocks of [512 x6, 128] columns
    (col = t*16 + b).
  * The GEMM runs entirely in FP16 (full PE rate, half the DMA/SBUF of
    f32r) with an error-compensated 3-term split
        x@w = xh@wh + xl@wh + xh@wl,
    xh = fp16(x), xl = fp16(x - xh): 11-bit factors multiply exactly into
    the fp32 accumulator, so the scheme carries ~22 effective mantissa
    bits; measured 0/256000 spike flips vs the fp32 reference.
  * w1 is stored m-major in DRAM ((HC*128, KF*128) with row m*128+p
    holding contraction-partition p of output chunk m) so each m-block's
    weights arrive in one contiguous-per-partition DMA and block 0
    starts computing as soon as the first chunks land.
  * BOTH LIF scans run fused on the Vector engine with scaled states
    M = beta*mem:
        A:  M_t = (V_{t-1} * -beta) + beta*cur_t        (scalar_tensor_tensor)
        B:  V_t = (M_t > beta) - M_t                    (scalar_tensor_tensor)
    The layer-1 step s operates on columns 0:128 (8 h-chunks x 16 batch)
    of a (128, 144) tile; columns 128:144 carry the layer-2 recurrence
    for global step s-32 (its beta*cur2, staged into the same c1 tile by
    a Scalar copy after GEMM2).  One op pair advances both chains, so
    the layer-2 scan costs no extra Vector instructions; the final 32
    lagged steps run as standalone (10,16) ops after the loop.
    spk1_t = Sign(M_t - beta) on the Scalar engine, stored +-1 in f32.
    beta is folded into w1/b1/w2/b2 host-side.
  * Layer-2 currents use the sign-spike trick (spk@w2.T = s@(w2/2).T +
    rowsum(w2)/2) as a single full-precision fp32 GEMM (sign spikes are
    exact in any dtype), issued as 2 rounds of 4 column-tiled matmuls
    (M=10 output rows per 32-column PE group), so the PE runs 4 h-chunk
    matmuls concurrently.  The partition-group partials are summed by
    1 Scalar activation (+bias) and 3 Vector adds into c2.
  * spk2 = (m2 > beta) on Vector; mem2 = m2 * (1/beta) on Scalar; both
    stream out per block.
"""

import numpy as np

BETA = 0.95
B, T, I, H, O = 128, 200, 784, 1024, 10
NCORES = 8
BL = B // NCORES          # 16 batch rows per core
TB = T * BL               # 3200 (t-major, b-minor columns)
KF = 6                    # full 128-row contraction chunks (rows 0..767)
KT = 48                   # packed tail: [xh_t; xh_t; xl_t] x [w1h_t; w1l_t; w1h_t]
HC = H // 128             # 8 h-chunks
NBLK = (512, 512, 512, 512, 512, 512, 128)
CHUNK = 512
LAG = 32                  # layer-2 scan runs LAG steps behind layer-1
W1 = HC * BL              # 128 layer-1 columns per step
WF = W1 + BL              # 144 fused columns per step

_nc_cache = None


def _build():
    import concourse.bacc as bacc
    import concourse.mybir as mybir
    from concourse.tile import TileContext

    Alu = mybir.AluOpType
    Act = mybir.ActivationFunctionType
    f32 = mybir.dt.float32
    f16 = mybir.dt.float16

    nc = bacc.Bacc("TRN2", target_bir_lowering=False, debug=False)

    xh_d = nc.dram_tensor("xh", (KF * 128, TB), f16, kind="ExternalInput")
    xl_d = nc.dram_tensor("xl", (KF * 128, TB), f16, kind="ExternalInput")
    xt_d = nc.dram_tensor("xt", (KT, TB), f16, kind="ExternalInput")
    # m-major: row m*128+p holds w1[k-partition p, h-chunk m], 768 cols (k)
    w1h_d = nc.dram_tensor("w1h", (HC * 128, KF * 128), f16, kind="ExternalInput")
    w1l_d = nc.dram_tensor("w1l", (HC * 128, KF * 128), f16, kind="ExternalInput")
    w1t_d = nc.dram_tensor("w1t", (KT, HC * 128), f16, kind="ExternalInput")
    b1c = nc.dram_tensor("b1c", (128, HC), f32, kind="ExternalInput")
    w2p_d = nc.dram_tensor("w2p", (128, HC * O), f32, kind="ExternalInput")
    b2c = nc.dram_tensor("b2c", (O, 1), f32, kind="ExternalInput")
    S2 = nc.dram_tensor("S2", (O, TB), f32, kind="ExternalOutput")
    M2 = nc.dram_tensor("M2", (O, TB), f32, kind="ExternalOutput")

    blocks = []
    c0 = 0
    for n in NBLK:
        blocks.append((c0, n))
        c0 += n
    assert c0 == TB
    starts = [c0 // BL for c0, _ in blocks]       # first global step per block
    nb = len(blocks)

    def step_block(s):
        for bi in range(nb - 1, -1, -1):
            if s >= starts[bi]:
                return bi
        raise AssertionError

    with TileContext(nc) as tc:
        with (
            tc.tile_pool(name="const", bufs=1) as cpool,
            tc.tile_pool(name="l2", bufs=1) as l2pool,
            tc.tile_pool(name="c1b", bufs=2) as c1pool,
            tc.tile_pool(name="xt", bufs=3) as xpool,
            tc.tile_pool(name="mv", bufs=2) as mvpool,
            tc.tile_pool(name="o2", bufs=2) as opool,
            tc.tile_pool(name="ps1", bufs=4, space="PSUM") as ps1,
            tc.tile_pool(name="ps2", bufs=2, space="PSUM") as ps2,
        ):
            w1h_sb = cpool.tile([128, HC, KF * 128], f16)
            w1l_sb = cpool.tile([128, HC, KF * 128], f16)
            w1t_sb = cpool.tile([KT, HC * 128], f16)
            b1_sb = cpool.tile([128, HC], f32)
            w2_sb = cpool.tile([128, HC * O], f32)
            b2_sb = cpool.tile([O, 1], f32)

            # Block-0 x tiles, DMA'd per k-chunk interleaved with the
            # per-m weight chunks so m_block(m) finds its inputs resident.
            xh0 = xpool.tile([128, KF, CHUNK], f16, tag="xh", name="xh0")
            xl0 = xpool.tile([128, KF, CHUNK], f16, tag="xl", name="xl0")
            xt0 = xpool.tile([KT, CHUNK], f16, tag="xt", name="xt0")
            n0 = blocks[0][1]
            for k in range(KF):
                nc.sync.dma_start(
                    out=xh0[:, k, :n0], in_=xh_d[k * 128:(k + 1) * 128, 0:n0]
                )
                nc.sync.dma_start(out=w1h_sb[:, k], in_=w1h_d[k * 128:(k + 1) * 128])
                nc.sync.dma_start(
                    out=xl0[:, k, :n0], in_=xl_d[k * 128:(k + 1) * 128, 0:n0]
                )
                nc.sync.dma_start(out=w1l_sb[:, k], in_=w1l_d[k * 128:(k + 1) * 128])
            nc.sync.dma_start(out=b1_sb[:], in_=b1c[:])
            nc.sync.dma_start(out=xt0[:, :n0], in_=xt_d[:, 0:n0])
            nc.sync.dma_start(out=w1t_sb[:], in_=w1t_d[:])
            for m in range(KF, HC):
                nc.sync.dma_start(out=w1h_sb[:, m], in_=w1h_d[m * 128:(m + 1) * 128])
                nc.sync.dma_start(out=w1l_sb[:, m], in_=w1l_d[m * 128:(m + 1) * 128])
            nc.sync.dma_start(out=w2_sb[:], in_=w2p_d[:])
            nc.sync.dma_start(out=b2_sb[:], in_=b2c[:])

            c2 = l2pool.tile([O, TB], f32)       # beta*cur2 (read-only after combine)
            m2tail = l2pool.tile([O, LAG * BL], f32)   # beta*mem2, last LAG steps

            negbeta = cpool.tile([128, 1], f32)
            nc.vector.memset(negbeta[:], -BETA)

            v1 = mvpool.tile([128, WF], f32, tag="v1")
            nc.vector.memset(v1[:], 0.0)

            c1_tiles = {}
            spk_tiles = {}

            def gemm1(bi):
                c0, n = blocks[bi]
                nt = n // BL
                if bi == 0:
                    xh, xl, xt = xh0, xl0, xt0
                else:
                    xh = xpool.tile([128, KF, CHUNK], f16, tag="xh")
                    xl = xpool.tile([128, KF, CHUNK], f16, tag="xl")
                    xt = xpool.tile([KT, CHUNK], f16, tag="xt")
                    for k in range(KF):
                        nc.sync.dma_start(
                            out=xh[:, k, :n],
                            in_=xh_d[k * 128:(k + 1) * 128, c0:c0 + n],
                        )
                        nc.sync.dma_start(
                            out=xl[:, k, :n],
                            in_=xl_d[k * 128:(k + 1) * 128, c0:c0 + n],
                        )
                    nc.sync.dma_start(out=xt[:, :n], in_=xt_d[:, c0:c0 + n])
                c1 = c1pool.tile([128, 32, WF], f32, tag="c1")
                c1_tiles[bi] = c1
                # zero the layer-2 lanes: keeps block 0's v2 state exactly
                # 0 until the first real layer-2 step enters at s = LAG,
                # and avoids uninitialized-SBUF reads in the fused ops.
                nc.vector.memset(c1[:, :, W1:WF], 0.0)
                spk = c1pool.tile([128, HC, 32, BL], f32, tag="spk", name="spk")
                spk_tiles[bi] = spk
                for m in range(HC):
                    p1 = ps1.tile([128, CHUNK], f32, tag="p1")
                    i = 0
                    for k in range(KF):
                        for (wt, xs_) in (
                            (w1h_sb, xh), (w1h_sb, xl), (w1l_sb, xh),
                        ):
                            nc.tensor.matmul(
                                p1[:, :n],
                                lhsT=wt[:, m, k * 128:(k + 1) * 128],
                                rhs=xs_[:, k, :n],
                                start=(i == 0),
                                stop=False,
                            )
                            i += 1
                    nc.tensor.matmul(
                        p1[:, :n],
                        lhsT=w1t_sb[:, m * 128:(m + 1) * 128],
                        rhs=xt[:, :n],
                        start=False,
                        stop=True,
                    )
                    p1v = p1.rearrange("p (t b) -> p t b", b=BL)
                    nc.scalar.activation(
                        out=c1[:, :nt, m * BL:(m + 1) * BL],
                        in_=p1v[:, :nt, :],
                        func=Act.Identity,
                        bias=b1_sb[:, m:m + 1],
                        scale=1.0,
                    )

            def gemm2(bi):
                c0, n = blocks[bi]
                spk = spk_tiles.pop(bi)
                p2 = ps2.tile([128, CHUNK], f32, tag="p2")
                spk2d = spk.rearrange("p c t b -> p (c t b)")
                for r in range(2):
                    for j in range(4):
                        c = r * 4 + j
                        nc.tensor.matmul(
                            p2[32 * j:32 * j + O, :n],
                            lhsT=w2_sb[:, c * O:(c + 1) * O],
                            rhs=spk2d[:, c * 32 * BL:c * 32 * BL + n],
                            start=(r == 0),
                            stop=(r == 1),
                            tile_position=(0, 32 * j),
                        )
                # combine the 4 partition-group partials + bias into c2
                nc.scalar.activation(
                    out=c2[:, c0:c0 + n],
                    in_=p2[0:O, :n],
                    func=Act.Identity,
                    bias=b2_sb[:, 0:1],
                    scale=1.0,
                )
                for j in range(1, 4):
                    nc.vector.tensor_tensor(
                        out=c2[:, c0:c0 + n],
                        in0=p2[32 * j:32 * j + O, :n],
                        in1=c2[:, c0:c0 + n],
                        op=Alu.add,
                    )
                # stage beta*cur2 of steps [g0, g1) into the layer-2 lanes
                # of the c1 tiles that will process them (at s = g + LAG).
                g0, g1 = c0 // BL, (c0 + n) // BL
                g = g0
                while g < g1:
                    s = g + LAG
                    if s >= T:
                        break                    # tail: consumed standalone
                    bj = step_block(s)
                    cnt = min(g1 - g, starts[bj] + blocks[bj][1] // BL - s)
                    dst = c1_tiles[bj]
                    nc.scalar.activation(
                        out=dst[0:O, s - starts[bj]:s - starts[bj] + cnt, W1:WF],
                        in_=c2[:, g * BL:(g + cnt) * BL]
                        .rearrange("p (t b) -> p t b", b=BL),
                        func=Act.Identity, bias=0.0, scale=1.0,
                    )
                    g += cnt

            def scans(bi):
                """Fused layer-1/layer-2 scan over block bi's steps.
                The A-op updates c1[:, tl] IN PLACE (currents -> M)."""
                nonlocal v1
                c1 = c1_tiles[bi]
                spk = spk_tiles[bi]
                for tl in range(blocks[bi][1] // BL):
                    m1 = c1[:, tl]
                    nc.vector.scalar_tensor_tensor(
                        out=m1, in0=v1[:], scalar=-BETA, in1=m1,
                        op0=Alu.mult, op1=Alu.add,
                    )
                    v1n = mvpool.tile([128, WF], f32, tag="v1")
                    nc.vector.scalar_tensor_tensor(
                        out=v1n[:], in0=m1, scalar=BETA, in1=m1,
                        op0=Alu.is_gt, op1=Alu.subtract,
                    )
                    # sign-spikes s = 2*spk-1 on the Scalar engine; the
                    # (s+1)/2 un-mapping is folded into w2/2 + bias rowsum.
                    nc.scalar.activation(
                        spk[:, :, tl, :],
                        c1[:, tl, 0:W1].rearrange("p (c b) -> p c b", b=BL),
                        Act.Sign,
                        bias=negbeta[:, 0:1], scale=1.0,
                    )
                    v1 = v1n

            def scan2_tail():
                """Standalone layer-2 steps g = T-LAG .. T-1."""
                v2 = v1[0:O, W1:WF]
                for i, g in enumerate(range(T - LAG, T)):
                    ms = m2tail[:, i * BL:(i + 1) * BL]
                    nc.vector.scalar_tensor_tensor(
                        out=ms, in0=v2, scalar=-BETA,
                        in1=c2[:, g * BL:(g + 1) * BL],
                        op0=Alu.mult, op1=Alu.add,
                    )
                    v2n = mvpool.tile([O, BL], f32, tag="v2")
                    nc.vector.scalar_tensor_tensor(
                        out=v2n[:], in0=ms, scalar=BETA, in1=ms,
                        op0=Alu.is_gt, op1=Alu.subtract,
                    )
                    v2 = v2n[:]

            def out2(bi):
                """spk2 (Vector) + 1/beta un-scale (Scalar) for block bi,
                reading beta*mem2 from the fused-scan slots in the c1
                tiles (and m2tail for the last LAG steps)."""
                c0, n = blocks[bi]
                g0, g1 = c0 // BL, (c0 + n) // BL
                s2b = opool.tile([O, CHUNK], f32, tag="s2b")
                m2s = opool.tile([O, CHUNK], f32, tag="m2s")
                g = g0
                while g < g1:
                    if g + LAG < T:
                        s = g + LAG
                        bj = step_block(s)
                        cnt = min(g1 - g, starts[bj] + blocks[bj][1] // BL - s,
                                  T - LAG - g)
                        src = c1_tiles[bj][0:O, s - starts[bj]:
                                           s - starts[bj] + cnt, W1:WF]
                    else:
                        cnt = g1 - g
                        i = g - (T - LAG)
                        src = (m2tail[:, i * BL:(i + cnt) * BL]
                               .rearrange("p (t b) -> p t b", b=BL))
                    lo, hi = (g - g0) * BL, (g - g0 + cnt) * BL
                    nc.vector.tensor_scalar(
                        s2b[:, lo:hi].rearrange("p (t b) -> p t b", b=BL),
                        src, BETA, None, Alu.is_gt,
                    )
                    nc.scalar.activation(
                        out=m2s[:, lo:hi].rearrange("p (t b) -> p t b", b=BL),
                        in_=src,
                        func=Act.Identity, bias=0.0, scale=1.0 / BETA,
                    )
                    g += cnt
                nc.sync.dma_start(out=S2[:, c0:c0 + n], in_=s2b[:, :n])
                nc.sync.dma_start(out=M2[:, c0:c0 + n], in_=m2s[:, :n])

            for bi in range(nb):
                gemm1(bi)
                if bi > 0:
                    gemm2(bi - 1)
                scans(bi)
                if bi > 1:
                    out2(bi - 2)       # its m2 lives in mblk(bi-1) + earlier
            gemm2(nb - 1)
            scan2_tail()
            out2(nb - 2)
            out2(nb - 1)
            c1_tiles.clear()

    nc.compile()
    return nc


def _get_nc():
    global _nc_cache
    if _nc_cache is None:
        _nc_cache = _build()
    return _nc_cache


def _split16(a):
    hi = np.asarray(a, np.float16)
    lo = np.asarray(a - hi.astype(np.float32), np.float16)
    return hi, lo


def _prep_shared(w1, b1, w2, b2):
    w1s = (BETA * w1).T.astype(np.float32)        # (784, 1024)
    w1h_f, w1l_f = _split16(w1s)

    # m-major weight layout: row m*128+p, col k*128+c = w1s[k*128+p, m*128+c]
    def mmajor(wf):
        return np.ascontiguousarray(
            wf[:768].reshape(KF, 128, HC, 128)      # (k, p, m, c)
            .transpose(2, 1, 0, 3)                  # (m, p, k, c)
            .reshape(HC * 128, KF * 128)
        )
    w1h = mmajor(w1h_f)
    w1l = mmajor(w1l_f)
    # packed 48-row tail: rows pair as (w1h,xh), (w1l,xh), (w1h,xl)
    w1t = np.ascontiguousarray(
        np.concatenate([w1h_f[768:], w1l_f[768:], w1h_f[768:]], axis=0)
    )
    b1v = np.ascontiguousarray((BETA * b1).astype(np.float32).reshape(HC, 128).T)
    # GEMM2 consumes sign-spikes s = 2*spk-1:  spk@w2.T = s@(w2/2).T + rowsum(w2)/2
    w2s = (0.5 * BETA * w2).T.astype(np.float32).reshape(HC, 128, O).transpose(1, 0, 2)
    w2p = np.ascontiguousarray(w2s.reshape(128, HC * O))
    b2v = (BETA * (b2 + 0.5 * w2.sum(axis=1))).astype(np.float32).reshape(O, 1)
    return w1h, w1l, w1t, b1v, w2p, b2v


def _make_in_maps(x, w1, b1, w2, b2):
    w1h, w1l, w1t, b1v, w2p, b2v = _prep_shared(w1, b1, w2, b2)
    in_maps = []
    for c in range(NCORES):
        xs = x[c * BL:(c + 1) * BL]                     # (BL, T, I)
        xT = np.ascontiguousarray(
            xs.transpose(2, 1, 0).reshape(I, TB)        # col = t*BL + b
        )
        xh_f, xl_f = _split16(xT)
        xh = np.ascontiguousarray(xh_f[:768])
        xl = np.ascontiguousarray(xl_f[:768])
        xt = np.ascontiguousarray(
            np.concatenate([xh_f[768:], xh_f[768:], xl_f[768:]], axis=0)
        )
        in_maps.append({
            "xh": xh, "xl": xl, "xt": xt, "w1h": w1h, "w1l": w1l, "w1t": w1t,
            "b1c": b1v, "w2p": w2p, "b2c": b2v,
        })
    return in_maps


def kernel(x, w1, b1, w2, b2):
    from concourse.bass_utils import run_bass_kernel_spmd

    nc = _get_nc()
    in_maps = _make_in_maps(x, w1, b1, w2, b2)
    res = run_bass_kernel_spmd(nc, in_maps, core_ids=list(range(NCORES)))

    spk = np.empty((B, T, O), np.float32)
    mem = np.empty((B, T, O), np.float32)
    for c in range(NCORES):
        r = res.results[c]
        spk[c * BL:(c + 1) * BL] = r["S2"].reshape(O, T, BL).transpose(2, 1, 0)
        mem[c * BL:(c + 1) * BL] = r["M2"].reshape(O, T, BL).transpose(2, 1, 0)
    return spk, mem
